# revision 1
# baseline (speedup 1.0000x reference)
"""Causal single-head attention (b=4, n=2048, d=1024) on 8 trn2 cores.

Sharding: 2 cores per batch element. Each batch's 16 query blocks (128
rows) are assigned to its core pair so that every core processes one
q-block at each "capacity" in {2,4,...,16} key-blocks: even-parity
cores take even-index q-blocks (odd causal limit), odd-parity cores
take odd-index ones (even causal limit). Odd causal limits waste one
fully-masked 128-key block; total per-core key-block visits = 72
(vs 68 ideal) and the instruction stream is identical on all cores
(pure SPMD) — only the data (gathered q rows + mask) differs.

Per core: K^T/V/Q^T projections (PE), scores = Q^T·K per q-block,
masked softmax (DVE reduce + ACT exp), PE transpose of the weights,
AV accumulation, 1/rowsum folded into the PSUM->SBUF copyback.
The 1/sqrt(d) score scale (2^-5, exact) is folded into Q^T.
"""

import numpy as np

P = 128
B, N, D = 4, 2048, 1024
NCORES = 8
CAPS = (16, 14, 12, 10, 8, 6, 4, 2)  # key-block capacity per slot
NEG = -1.0e30

# Matmul compute dtype: "f32" (exact, 4 cyc/row) or "f32r" (full rate,
# TF32-ish hardware numerics).
MM_DT = "f32r"

_prog_cache = {}


def _split_multi_waits(nc, max_waits=1):
    """walrus in this container rejects more than one sem wait per
    instruction ("Too many sync wait commands"). After Tile scheduling,
    hoist extra waits onto same-engine nops inserted just before the
    instruction (same blocking semantics: engine queues are in-order)."""
    from concourse import mybir

    n = 0
    for fn in nc.m.functions:
        for bb in fn.blocks:
            out = []
            for ins in bb.instructions:
                si = ins.sync_info
                waits = list(si.on_wait) if si and si.on_wait else []
                if len(waits) > max_waits:
                    extra = waits[:-max_waits]
                    si.on_wait = waits[-max_waits:]
                    for j in range(0, len(extra), max_waits):
                        nop = mybir.InstNoOp(
                            name=f"waitsplit_{n}", ins=[], outs=[],
                            engine=ins.engine)
                        n += 1
                        nop.sync_info = mybir.SyncInfo(
                            on_wait=extra[j:j + max_waits], on_update=[])
                        out.append(nop)
                out.append(ins)
            bb.instructions[:] = out


def _build_program(mm_dt_name):
    import concourse.bass as bass
    import concourse.tile as tile
    from concourse import mybir
    from concourse.masks import make_identity

    f32 = mybir.dt.float32
    mmdt = f32 if mm_dt_name == "f32" else mybir.dt.float32r

    nc = bass.Bass("TRN2", target_bir_lowering=False, debug=False,
                   num_devices=NCORES, dynamic_dma_scratch_size=2048)

    xqT_d = nc.dram_tensor("xqT", [D, 8 * P], mmdt, kind="ExternalInput").ap()
    xkT_d = nc.dram_tensor("xkT", [D, N], mmdt, kind="ExternalInput").ap()
    wq_d = nc.dram_tensor("wq", [D, D], mmdt, kind="ExternalInput").ap()
    wk_d = nc.dram_tensor("wk", [D, D], mmdt, kind="ExternalInput").ap()
    wv_d = nc.dram_tensor("wv", [D, D], mmdt, kind="ExternalInput").ap()
    mask_d = nc.dram_tensor("mask", [P, 2 * P], f32, kind="ExternalInput").ap()
    out_d = nc.dram_tensor("out", [8 * P, D], f32, kind="ExternalOutput").ap()

    DC = D // P  # 8 contraction chunks
    xqT_r = xqT_d.rearrange("(dc p) q -> p dc q", p=P)
    xkT_r = xkT_d.rearrange("(dc p) k -> p dc k", p=P)
    wq_r = wq_d.rearrange("(dc p) e -> p dc e", p=P)
    wk_r = wk_d.rearrange("(dc p) e -> p dc e", p=P)
    wv_r = wv_d.rearrange("(dc p) e -> p dc e", p=P)

    with tile.TileContext(nc) as tc:
        import contextlib
        with contextlib.ExitStack() as ctx:
            cpool = ctx.enter_context(tc.tile_pool(name="cpool", bufs=1))
            qtp = ctx.enter_context(tc.tile_pool(name="qtp", bufs=1))
            ktp = ctx.enter_context(tc.tile_pool(name="ktp", bufs=1))
            vp = ctx.enter_context(tc.tile_pool(name="vp", bufs=1))

            ident_f = cpool.tile([P, P], f32, name="ident_f")
            make_identity(nc, ident_f)
            ident = cpool.tile([P, P], mmdt, name="ident")
            nc.vector.tensor_copy(ident[:], ident_f[:])
            mask_sb = cpool.tile([P, 2 * P], f32, name="mask_sb")
            nc.sync.dma_start(mask_sb[:], mask_d)

            QT = qtp.tile([P, DC, 8 * P], mmdt, name="QT")
            KT = ktp.tile([P, DC, N], mmdt, name="KT")
            V = vp.tile([P, N // P, D], mmdt, name="V")

            # ---- projections ----
            # Weights stream as four [P, 2, D] quarters (8KB/partition)
            # through 5 shared slots so the next phase's weights prefetch
            # into free slots while the current phase computes.
            with tc.tile_pool(name="wpool", bufs=5) as wpool, \
                 tc.tile_pool(name="xpool", bufs=2) as xpool, \
                 tc.tile_pool(name="ppj", bufs=4, space="PSUM") as ppj:

                def load_w(src_r, nm):
                    qs = []
                    for i in range(4):
                        t = wpool.tile([P, 2, D], mmdt, tag="w",
                                       name=f"{nm}_q{i}")
                        nc.sync.dma_start(t[:], src_r[:, 2 * i:2 * i + 2, :])
                        qs.append(t)
                    return qs

                # Q^T[e, q] = sum_d Wq[d, e] * xqT[d, q], scaled by 1/32
                # first x slice is DMA'd before the weights so the PE can
                # start as soon as the first weight quarters land
                xs0 = xpool.tile([P, DC, 256], mmdt, tag="xs", name="xs_q0")
                nc.sync.dma_start(xs0[:], xqT_r[:, :, 0:256])
                wq2 = load_w(wq_r, "wq")
                for qt in range(4):
                    if qt == 0:
                        xs = xs0
                    else:
                        xs = xpool.tile([P, DC, 256], mmdt, tag="xs",
                                        name="xs_q")
                        nc.sync.dma_start(
                            xs[:], xqT_r[:, :, qt * 256:(qt + 1) * 256])
                    for ec in range(DC):
                        ps = ppj.tile([P, 512], f32, tag="pj", name="ps_q")
                        for dc in range(DC):
                            nc.tensor.matmul(
                                ps[:, :256],
                                wq2[dc // 2][:, dc % 2, ec * P:(ec + 1) * P],
                                xs[:, dc, :],
                                start=(dc == 0), stop=(dc == DC - 1))
                        nc.vector.tensor_scalar_mul(
                            QT[:, ec, qt * 256:(qt + 1) * 256],
                            ps[:, :256], 1.0 / 32.0)

                # K^T[e, k] = sum_d Wk[d, e] * xkT[d, k]
                wk2 = load_w(wk_r, "wk")
                for kt in range(8):
                    xs = xpool.tile([P, DC, 256], mmdt, tag="xs", name="xs_k")
                    nc.sync.dma_start(xs[:], xkT_r[:, :, kt * 256:(kt + 1) * 256])
                    for ec in range(DC):
                        ps = ppj.tile([P, 512], f32, tag="pj", name="ps_k")
                        for dc in range(DC):
                            nc.tensor.matmul(
                                ps[:, :256],
                                wk2[dc // 2][:, dc % 2, ec * P:(ec + 1) * P],
                                xs[:, dc, :],
                                start=(dc == 0), stop=(dc == DC - 1))
                        nc.vector.tensor_copy(
                            KT[:, ec, kt * 256:(kt + 1) * 256], ps[:, :256])

                # V[k, e] = sum_d xkT[d, k] * Wv[d, e]
                wv2 = load_w(wv_r, "wv")
                for kp in range(N // 256):
                    xs = xpool.tile([P, DC, 256], mmdt, tag="xs", name="xs_v")
                    nc.sync.dma_start(xs[:], xkT_r[:, :, kp * 256:(kp + 1) * 256])
                    for half in range(2):
                        kc = 2 * kp + half
                        for h in range(2):
                            ps = ppj.tile([P, 512], f32, tag="pj", name="ps_v")
                            for dc in range(DC):
                                nc.tensor.matmul(
                                    ps,
                                    xs[:, dc, half * P:(half + 1) * P],
                                    wv2[dc // 2][:, dc % 2,
                                                 h * 512:(h + 1) * 512],
                                    start=(dc == 0), stop=(dc == DC - 1))
                            nc.vector.tensor_copy(
                                V[:, kc, h * 512:(h + 1) * 512], ps)

            # ---- attention, software-pipelined over the 8 slots ----
            with tc.tile_pool(name="scp", bufs=3) as scp, \
                 tc.tile_pool(name="wtp", bufs=2) as wtp, \
                 tc.tile_pool(name="obp", bufs=2) as obp, \
                 tc.tile_pool(name="stp", bufs=3) as stp, \
                 tc.tile_pool(name="psc", bufs=2, space="PSUM") as psc, \
                 tc.tile_pool(name="pav", bufs=4, space="PSUM") as pav, \
                 tc.tile_pool(name="ptr", bufs=2, space="PSUM") as ptr:

                scores = [None] * len(CAPS)
                stats = [None] * len(CAPS)

                def emit_scores(slot):
                    s = CAPS[slot]
                    L = P * s
                    sc = scp.tile([P, N], mmdt, tag="sc", name=f"sc{slot}")
                    st = stp.tile([P, 4], f32, tag="st", name=f"st{slot}")
                    scores[slot] = sc
                    stats[slot] = st
                    off = 0
                    widths = [512] * (L // 512) + ([256] if L % 512 else [])
                    for w in widths:
                        ps = psc.tile([P, 512], f32, tag="psc", name=f"pssc{slot}")
                        for ec in range(DC):
                            nc.tensor.matmul(
                                ps[:, :w],
                                QT[:, ec, slot * P:(slot + 1) * P],
                                KT[:, ec, off:off + w],
                                start=(ec == 0), stop=(ec == DC - 1))
                        end = off + w
                        if end == L:
                            if w == 512:
                                nc.vector.tensor_copy(
                                    sc[:, off:off + 256], ps[:, 0:256])
                            nc.vector.tensor_add(
                                sc[:, L - 256:L], ps[:, w - 256:w], mask_sb[:])
                        else:
                            nc.vector.tensor_copy(sc[:, off:end], ps[:, :w])
                        off = end
                    # softmax stats + in-place exp
                    nc.vector.tensor_reduce(
                        st[:, 0:1], sc[:, :L], axis=mybir.AxisListType.X,
                        op=mybir.AluOpType.max, negate=True)
                    nc.scalar.activation(
                        sc[:, :L], sc[:, :L], mybir.ActivationFunctionType.Exp,
                        bias=st[:, 0:1], scale=1.0, accum_out=st[:, 1:2])
                    nc.vector.reciprocal(st[:, 2:3], st[:, 1:2])

                def emit_av(slot):
                    s = CAPS[slot]
                    sc = scores[slot]
                    st = stats[slot]
                    wt = wtp.tile([P, N // P, P], mmdt, tag="wt", name=f"wt{slot}")
                    for j in range(s):
                        pt = ptr.tile([P, P], mmdt, tag="ptr", name=f"pt{slot}")
                        nc.tensor.transpose(pt, sc[:, j * P:(j + 1) * P], ident)
                        nc.vector.tensor_copy(wt[:, j, :], pt)
                    avs = []
                    for h in range(2):
                        av = pav.tile([P, 512], f32, tag="pav", name=f"av{slot}_{h}")
                        avs.append(av)
                    for j in range(s):
                        for h in range(2):
                            nc.tensor.matmul(
                                avs[h],
                                wt[:, j, :],
                                V[:, j, h * 512:(h + 1) * 512],
                                start=(j == 0), stop=(j == s - 1))
                    ob = obp.tile([P, D], f32, tag="ob", name=f"ob{slot}")
                    for h in range(2):
                        nc.vector.tensor_scalar_mul(
                            ob[:, h * 512:(h + 1) * 512], avs[h], st[:, 2:3])
                    nc.sync.dma_start(out_d[slot * P:(slot + 1) * P, :], ob)

                emit_scores(0)
                emit_scores(1)
                for b_ in range(len(CAPS)):
                    if b_ + 2 < len(CAPS):
                        emit_scores(b_ + 2)
                    emit_av(b_)

    _split_multi_waits(nc)
    return nc


def _host_prep(x, Wq, Wk, Wv):
    """Build per-core input maps."""
    x = np.ascontiguousarray(x, dtype=np.float32)
    tri = np.where(
        np.arange(P)[None, :] <= np.arange(P)[:, None], 0.0, NEG
    ).astype(np.float32)
    mask_even = np.concatenate(  # parity 0: diag block then fully-masked block
        [tri, np.full((P, P), NEG, np.float32)], axis=1)
    mask_odd = np.concatenate(  # parity 1: fully-visible block then diag block
        [np.zeros((P, P), np.float32), tri], axis=1)

    in_maps = []
    for c in range(NCORES):
        bi, r = c // 2, c % 2
        rbs = [s - 2 + r for s in CAPS]
        xq = np.concatenate([x[bi, rb * P:(rb + 1) * P, :] for rb in rbs], axis=0)
        in_maps.append({
            "xqT": np.ascontiguousarray(xq.T),
            "xkT": np.ascontiguousarray(x[bi].T),
            "wq": np.ascontiguousarray(Wq, dtype=np.float32),
            "wk": np.ascontiguousarray(Wk, dtype=np.float32),
            "wv": np.ascontiguousarray(Wv, dtype=np.float32),
            "mask": mask_odd if r else mask_even,
        })
    return in_maps


def _host_gather(results):
    out = np.empty((B, N, D), dtype=np.float32)
    for c in range(NCORES):
        bi, r = c // 2, c % 2
        res = results[c]["out"]
        for k, s in enumerate(CAPS):
            rb = s - 2 + r
            out[bi, rb * P:(rb + 1) * P, :] = res[k * P:(k + 1) * P, :]
    return out


def kernel(x, Wq, Wk, Wv, _trace=False, _trace_kwargs=None):
    from concourse.bass_utils import run_bass_kernel_spmd

    key = MM_DT
    if key not in _prog_cache:
        _prog_cache[key] = _build_program(key)
    nc = _prog_cache[key]

    in_maps = _host_prep(x, Wq, Wk, Wv)
    kw = dict(_trace_kwargs or {})
    res = run_bass_kernel_spmd(nc, in_maps, list(range(NCORES)),
                               trace=_trace, **kw)
    out = _host_gather(res.results)
    if _trace:
        return out, res
    return out



# revision 13
# speedup vs baseline: 1.3488x; 1.3488x over previous
"""Causal single-head attention (b=4, n=2048, d=1024) on 8 trn2 cores.

Sharding: 2 cores per batch element, with the baseline's parity trick
for the q-blocks (slot t covers q-block rb = CAPS[t]-2+parity so every
core sees a uniform capacity ladder 16,14,...,2 and the instruction
stream is pure SPMD).

vs the f32r baseline, this version:
- runs every matmul in bf16 (1 cyc/row at any free size; 2-byte
  weights double-buffer in the PE so LDWEIGHTS hides under compute,
  where the f32r baseline lost ~35-60ns per matmul).
- computes scores TRANSPOSED (S^T[k,q] = K^T~.T @ Q^T) so the AV
  matmul needs no PE transposes, softmax needs no row-max pass
  (logits are ~N(0,1) after the folded 1/32 scale; exp() without a
  max-shift cannot overflow f32), and row-sums come from a 1-column
  ones matmul accumulated alongside AV.
- applies the causal mask as a 9th PSUM-accumulation step on the PE
  (identity-stationary x mask-moving) instead of DVE adds.
- tensor-parallel splits the K/V projections along d_out across each
  core pair, exchanged with a pair AllGather through DRAM (MODE="cc");
  MODE="dup" falls back to computing both halves locally.
- pairs q-slots (16,14),(12,10),(8,6),(4,2) so common key-blocks are
  processed with 256-wide moving operands.
"""

import numpy as np

P = 128
B, N, D = 4, 2048, 1024
NCORES = 8
DC = D // P  # 8 contraction chunks
CAPS = (16, 14, 12, 10, 8, 6, 4, 2)
SPAIRS = ((16, 14), (12, 10), (8, 6), (4, 2))  # (sA, sB); slots 2i, 2i+1
NEG = -1.0e9
MODE = "cc"  # "cc": pair-AllGather K/V halves; "dup": duplicate K/V
DEBUG_WT = False  # also emit per-pair exp(score) tiles to a debug output
GROUPS = [[0, 1], [2, 3], [4, 5], [6, 7]]
MM_DT = "bf16"  # kept for test.py compat; ignored

_prog_cache = {}


def _split_multi_waits(nc, max_waits=1):
    """walrus in this container rejects more than one sem wait per
    instruction ("Too many sync wait commands"). After Tile scheduling,
    hoist extra waits onto same-engine nops inserted just before the
    instruction (same blocking semantics: engine queues are in-order)."""
    from concourse import mybir

    n = 0
    for fn in nc.m.functions:
        for bb in fn.blocks:
            out = []
            for ins in bb.instructions:
                si = ins.sync_info
                waits = list(si.on_wait) if si and si.on_wait else []
                if len(waits) > max_waits:
                    extra = waits[:-max_waits]
                    si.on_wait = waits[-max_waits:]
                    for j in range(0, len(extra), max_waits):
                        nop = mybir.InstNoOp(
                            name=f"waitsplit_{n}", ins=[], outs=[],
                            engine=ins.engine)
                        n += 1
                        nop.sync_info = mybir.SyncInfo(
                            on_wait=extra[j:j + max_waits], on_update=[])
                        out.append(nop)
                out.append(ins)
            bb.instructions[:] = out


def _build_program(mode):
    import contextlib

    import concourse.bass as bass
    import concourse.tile as tile
    from concourse import mybir
    from concourse.masks import make_identity

    f32 = mybir.dt.float32
    bf16 = mybir.dt.bfloat16
    cc = mode == "cc"
    EH = 4 if cc else 8  # e-chunks of K/V computed locally

    nc = bass.Bass("TRN2", target_bir_lowering=False, debug=False,
                   num_devices=NCORES, dynamic_dma_scratch_size=2048)

    xqT_d = nc.dram_tensor("xqT", [D, 8 * P], bf16, kind="ExternalInput").ap()
    xkT_d = nc.dram_tensor("xkT", [D, N], bf16, kind="ExternalInput").ap()
    wq_d = nc.dram_tensor("wq", [D, D], bf16, kind="ExternalInput").ap()
    wk_d = nc.dram_tensor("wk", [D, EH * P], bf16, kind="ExternalInput").ap()
    wv_d = nc.dram_tensor("wv", [D, EH * P], bf16, kind="ExternalInput").ap()
    # 4 mask blocks: [cm0|cm1] 256-wide (A-zeros | B-mask at j=sB-2,sB-1)
    # then [am0|am1] 128-wide (A-mask at j=sA-2,sA-1)
    mask_d = nc.dram_tensor("mask", [P, 768], bf16, kind="ExternalInput").ap()
    out_d = nc.dram_tensor("out", [8 * P, D], f32, kind="ExternalOutput").ap()
    wtdbg_d = None
    if DEBUG_WT:
        wtdbg_d = nc.dram_tensor(
            "wtdbg", [4, P, 16, 2 * P], bf16, kind="ExternalOutput").ap()

    if cc:
        stg_k_in = nc.dram_tensor("stg_k_in", [4, P, N], bf16, kind="Internal")
        stg_k_out = nc.dram_tensor(
            "stg_k_out", [2, 4, P, N], bf16, kind="Internal")
        stg_v_in = nc.dram_tensor(
            "stg_v_in", [16, P, 4 * P], bf16, kind="Internal")
        stg_v_out = nc.dram_tensor(
            "stg_v_out", [2, 16, P, 4 * P], bf16, kind="Internal")

    xqT_r = xqT_d.rearrange("(dc p) q -> p dc q", p=P)
    xkT_r = xkT_d.rearrange("(dc p) k -> p dc k", p=P)
    wq_r = wq_d.rearrange("(dc p) e -> p dc e", p=P)
    wk_r = wk_d.rearrange("(dc p) e -> p dc e", p=P)
    wv_r = wv_d.rearrange("(dc p) e -> p dc e", p=P)

    with tile.TileContext(nc) as tc, contextlib.ExitStack() as ctx:
        cpool = ctx.enter_context(tc.tile_pool(name="cpool", bufs=1))
        qtp = ctx.enter_context(tc.tile_pool(name="qtp", bufs=1))
        ktp = ctx.enter_context(tc.tile_pool(name="ktp", bufs=1))
        vp = ctx.enter_context(tc.tile_pool(name="vp", bufs=1))

        ident_f = cpool.tile([P, P], f32, name="ident_f")
        make_identity(nc, ident_f)
        ident = cpool.tile([P, P], bf16, name="ident")
        nc.vector.tensor_copy(ident[:], ident_f[:])
        mask_sb = cpool.tile([P, 768], bf16, name="mask_sb")
        nc.sync.dma_start(mask_sb[:], mask_d)
        ones_sb = cpool.tile([P, 1], bf16, name="ones_sb")
        nc.vector.memset(ones_sb[:], 1.0)
        stats = cpool.tile([P, 8], f32, name="stats")

        QT = qtp.tile([P, DC, 8 * P], bf16, name="QT")
        KT = ktp.tile([P, DC, N], bf16, name="KT")
        V = vp.tile([P, N // P, D], bf16, name="V")

        # ---- projections ----
        with tc.tile_pool(name="xpool", bufs=4) as xpool, \
             tc.tile_pool(name="wpool", bufs=1) as wpool, \
             tc.tile_pool(name="hpool", bufs=1) as hpool, \
             tc.tile_pool(name="ppj", bufs=4, space="PSUM") as ppj:

            wq_sb = wpool.tile([P, DC, D], bf16, name="wq_sb")
            wk_sb = wpool.tile([P, DC, EH * P], bf16, name="wk_sb")
            wv_sb = wpool.tile([P, DC, EH * P], bf16, name="wv_sb")
            nc.sync.dma_start(wk_sb[:], wk_r)
            xk = []
            for kt in range(4):
                t = xpool.tile([P, DC, 4 * P], bf16, tag="x", name=f"xk{kt}")
                nc.sync.dma_start(t[:], xkT_r[:, :, kt * 512:(kt + 1) * 512])
                xk.append(t)
            nc.sync.dma_start(wv_sb[:], wv_r)
            nc.sync.dma_start(wq_sb[:], wq_r)

            # K^T[e, k] (own e-half in cc mode)
            if cc:
                k_half = hpool.tile([P, 4, N], bf16, name="k_half")
                for ec in range(4):
                    for kt in range(4):
                        ps = ppj.tile([P, 512], f32, tag="pj", name="ps_k")
                        for dc in range(DC):
                            nc.tensor.matmul(
                                ps, wk_sb[:, dc, ec * P:(ec + 1) * P],
                                xk[kt][:, dc, :],
                                start=(dc == 0), stop=(dc == DC - 1))
                        nc.vector.tensor_copy(
                            k_half[:, ec, kt * 512:(kt + 1) * 512], ps)
                    nc.sync.dma_start(stg_k_in.ap()[ec], k_half[:, ec, :])
                nc.gpsimd.collective_compute(
                    "AllGather", mybir.AluOpType.bypass,
                    replica_groups=GROUPS,
                    ins=[stg_k_in.ap()], outs=[stg_k_out.ap()])
                for half in range(2):
                    for ec in range(4):
                        nc.sync.dma_start(
                            KT[:, half * 4 + ec, :], stg_k_out.ap()[half, ec])
            else:
                for ec in range(8):
                    for kt in range(4):
                        ps = ppj.tile([P, 512], f32, tag="pj", name="ps_k")
                        for dc in range(DC):
                            nc.tensor.matmul(
                                ps, wk_sb[:, dc, ec * P:(ec + 1) * P],
                                xk[kt][:, dc, :],
                                start=(dc == 0), stop=(dc == DC - 1))
                        nc.vector.tensor_copy(
                            KT[:, ec, kt * 512:(kt + 1) * 512], ps)

            # V[k, e] (own e-half in cc mode)
            if cc:
                v_half = hpool.tile([P, 16, 4 * P], bf16, name="v_half")
                for kc in range(16):
                    ps = ppj.tile([P, 512], f32, tag="pj", name="ps_v")
                    xt = xk[kc // 4]
                    ko = (kc % 4) * P
                    for dc in range(DC):
                        nc.tensor.matmul(
                            ps, xt[:, dc, ko:ko + P], wv_sb[:, dc, :],
                            start=(dc == 0), stop=(dc == DC - 1))
                    nc.vector.tensor_copy(v_half[:, kc, :], ps)
                    nc.sync.dma_start(stg_v_in.ap()[kc], v_half[:, kc, :])
                nc.gpsimd.collective_compute(
                    "AllGather", mybir.AluOpType.bypass,
                    replica_groups=GROUPS,
                    ins=[stg_v_in.ap()], outs=[stg_v_out.ap()])
                for kc in range(16):
                    for half in range(2):
                        nc.sync.dma_start(
                            V[:, kc, half * 512:(half + 1) * 512],
                            stg_v_out.ap()[half, kc])
            else:
                for kc in range(16):
                    xt = xk[kc // 4]
                    ko = (kc % 4) * P
                    for h in range(2):
                        ps = ppj.tile([P, 512], f32, tag="pj", name="ps_v")
                        for dc in range(DC):
                            nc.tensor.matmul(
                                ps, xt[:, dc, ko:ko + P],
                                wv_sb[:, dc, h * 512:(h + 1) * 512],
                                start=(dc == 0), stop=(dc == DC - 1))
                        nc.vector.tensor_copy(
                            V[:, kc, h * 512:(h + 1) * 512], ps)

            # Q^T[e, q] for this core's 8 q-blocks (full d_out)
            for qc in range(2):
                xq = xpool.tile([P, DC, 4 * P], bf16, tag="x", name="xq")
                nc.sync.dma_start(
                    xq[:], xqT_r[:, :, qc * 512:(qc + 1) * 512])
                for ec in range(8):
                    ps = ppj.tile([P, 512], f32, tag="pj", name="ps_q")
                    for dc in range(DC):
                        nc.tensor.matmul(
                            ps, wq_sb[:, dc, ec * P:(ec + 1) * P],
                            xq[:, dc, :],
                            start=(dc == 0), stop=(dc == DC - 1))
                    nc.scalar.activation(
                        QT[:, ec, qc * 512:(qc + 1) * 512], ps,
                        mybir.ActivationFunctionType.Copy)

        # ---- attention over 4 slot-pairs ----
        with tc.tile_pool(name="wtp", bufs=2) as wtp, \
             tc.tile_pool(name="obp", bufs=2) as obp, \
             tc.tile_pool(name="pst", bufs=2, space="PSUM") as pst, \
             tc.tile_pool(name="pav", bufs=4, space="PSUM") as pav, \
             tc.tile_pool(name="pdn", bufs=1, space="PSUM") as pdn:

            wts = [None] * 4
            dens = [None] * 4

            def emit_scores(i):
                sA, sB = SPAIRS[i]
                q0 = 2 * i * P  # pair's 256-wide q-range in QT
                wt = wtp.tile([P, 16, 2 * P], bf16, tag="wt", name=f"wt{i}")
                wts[i] = wt
                for j in range(sA):
                    wide = 2 * P if j < sB else P
                    ps = pst.tile([P, 2 * P], f32, tag="st", name=f"ps_s{i}")
                    # mask as an extra accumulation step (identity x mask)
                    if j == sB - 2:
                        mk = mask_sb[:, 0:256]
                    elif j == sB - 1:
                        mk = mask_sb[:, 256:512]
                    elif j == sA - 2:
                        mk = mask_sb[:, 512:640]
                    elif j == sA - 1:
                        mk = mask_sb[:, 640:768]
                    else:
                        mk = None
                    for ec in range(DC):
                        nc.tensor.matmul(
                            ps[:, :wide],
                            KT[:, ec, j * P:(j + 1) * P],
                            QT[:, ec, q0:q0 + wide],
                            start=(ec == 0),
                            stop=(ec == DC - 1 and mk is None))
                    if mk is not None:
                        nc.tensor.matmul(ps[:, :wide], ident[:], mk,
                                         start=False, stop=True)
                    nc.scalar.activation(
                        wt[:, j, 0:wide], ps[:, :wide],
                        mybir.ActivationFunctionType.Exp, scale=1.0 / 32.0)
                if DEBUG_WT:
                    nc.sync.dma_start(wtdbg_d[i], wt[:])

            def emit_av(i):
                sA, sB = SPAIRS[i]
                wt = wts[i]
                # one PSUM bank per denominator: a matmul group-start zeroes
                # the whole bank, so A and B groups must not share one
                den = [pdn.tile([P, 1], f32, tag="dnA", name=f"denA{i}"),
                       pdn.tile([P, 1], f32, tag="dnB", name=f"denB{i}")]
                dens[i] = den
                avs = [[None, None], [None, None]]  # [ab][h]
                for ab in range(2):
                    for h in range(2):
                        avs[ab][h] = pav.tile(
                            [P, 512], f32, tag="av", name=f"av{i}_{ab}{h}")
                for j in range(sA):
                    for ab in range(2):
                        s = (sA, sB)[ab]
                        if j >= s:
                            continue
                        wcol = ab * P
                        for h in range(2):
                            nc.tensor.matmul(
                                avs[ab][h], wt[:, j, wcol:wcol + P],
                                V[:, j, h * 512:(h + 1) * 512],
                                start=(j == 0), stop=(j == s - 1))
                        nc.tensor.matmul(
                            den[ab][:], wt[:, j, wcol:wcol + P],
                            ones_sb[:],
                            start=(j == 0), stop=(j == s - 1))
                ob = obp.tile([P, 2, D], f32, tag="ob", name=f"ob{i}")
                for ab in range(2):
                    sc = 2 * i + ab
                    nc.vector.reciprocal(
                        stats[:, sc:sc + 1], den[ab][:])
                    for h in range(2):
                        nc.vector.tensor_scalar_mul(
                            ob[:, ab, h * 512:(h + 1) * 512],
                            avs[ab][h], stats[:, sc:sc + 1])
                    nc.sync.dma_start(
                        out_d[sc * P:(sc + 1) * P, :], ob[:, ab, :])

            emit_scores(0)
            for i in range(4):
                if i + 1 < 4:
                    emit_scores(i + 1)
                emit_av(i)

    _split_multi_waits(nc)
    return nc


def _host_prep(x, Wq, Wk, Wv):
    """Build per-core input maps (bf16 device inputs)."""
    import ml_dtypes
    bf = ml_dtypes.bfloat16

    x = np.ascontiguousarray(x, dtype=np.float32)
    ki = np.arange(P)[:, None]
    qi = np.arange(P)[None, :]
    triT = np.where(ki <= qi, 0.0, NEG).astype(np.float32)  # keep k <= q
    zero = np.zeros((P, P), np.float32)
    dead = np.full((P, P), NEG, np.float32)
    # blocks [cm0 | cm1 | am0 | am1]; cm* carry A-zeros in cols 0:128
    mask_r = {
        # parity 0: causal limit is odd -> diag then fully-dead block
        0: np.concatenate(
            [zero, triT, zero, dead, triT, dead], axis=1),
        # parity 1: causal limit is even -> full block then diag
        1: np.concatenate(
            [zero, zero, zero, triT, zero, triT], axis=1),
    }

    in_maps = []
    for c in range(NCORES):
        bi, r = c // 2, c % 2
        ecs = slice(r * 512, (r + 1) * 512) if MODE == "cc" else slice(0, D)
        rbs = [s - 2 + r for s in CAPS]
        xq = np.concatenate(
            [x[bi, rb * P:(rb + 1) * P, :] for rb in rbs], axis=0)
        in_maps.append({
            "xqT": np.ascontiguousarray(xq.T).astype(bf),
            "xkT": np.ascontiguousarray(x[bi].T).astype(bf),
            "wq": np.ascontiguousarray(Wq).astype(bf),
            "wk": np.ascontiguousarray(Wk[:, ecs]).astype(bf),
            "wv": np.ascontiguousarray(Wv[:, ecs]).astype(bf),
            "mask": np.ascontiguousarray(mask_r[r]).astype(bf),
        })
    return in_maps


def _host_gather(results):
    out = np.empty((B, N, D), dtype=np.float32)
    for c in range(NCORES):
        bi, r = c // 2, c % 2
        res = results[c]["out"]
        for t, s in enumerate(CAPS):
            rb = s - 2 + r
            out[bi, rb * P:(rb + 1) * P, :] = res[t * P:(t + 1) * P, :]
    return out


def kernel(x, Wq, Wk, Wv, _trace=False, _trace_kwargs=None):
    from concourse.bass_utils import run_bass_kernel_spmd

    key = MODE
    if key not in _prog_cache:
        _prog_cache[key] = _build_program(key)
    nc = _prog_cache[key]

    in_maps = _host_prep(x, Wq, Wk, Wv)
    kw = dict(_trace_kwargs or {})
    res = run_bass_kernel_spmd(nc, in_maps, list(range(NCORES)),
                               trace=_trace, **kw)
    out = _host_gather(res.results)
    if _trace:
        return out, res
    return out


# revision 17
# speedup vs baseline: 1.4605x; 1.0828x over previous
"""Causal single-head attention (b=4, n=2048, d=1024) on 8 trn2 cores.

Sharding: 2 cores per batch element, with the baseline's parity trick
for the q-blocks (slot t covers q-block rb = CAPS[t]-2+parity so every
core sees a uniform capacity ladder 16,14,...,2 and the instruction
stream is pure SPMD).

vs the f32r baseline, this version:
- runs every matmul in bf16 (1 cyc/row at any free size; 2-byte
  weights double-buffer in the PE so LDWEIGHTS hides under compute,
  where the f32r baseline lost ~35-60ns per matmul).
- computes scores TRANSPOSED (S^T[k,q] = K^T~.T @ Q^T) so the AV
  matmul needs no PE transposes, softmax needs no row-max pass
  (logits are ~N(0,1) after the folded 1/32 scale; exp() without a
  max-shift cannot overflow f32), and row-sums come from a 1-column
  ones matmul accumulated alongside AV.
- applies the causal mask as a 9th PSUM-accumulation step on the PE
  (identity-stationary x mask-moving) instead of DVE adds.
- tensor-parallel splits the K/V projections along d_out across each
  core pair, exchanged with a pair AllGather through DRAM (MODE="cc");
  MODE="dup" falls back to computing both halves locally.
- pairs q-slots (16,14),(12,10),(8,6),(4,2) so common key-blocks are
  processed with 256-wide moving operands.
"""

import numpy as np

P = 128
B, N, D = 4, 2048, 1024
NCORES = 8
DC = D // P  # 8 contraction chunks
CAPS = (16, 14, 12, 10, 8, 6, 4, 2)
SPAIRS = ((16, 14), (12, 10), (8, 6), (4, 2))  # (sA, sB); slots 2i, 2i+1
NEG = -1.0e9
MODE = "cc"  # "cc": pair-AllGather K/V halves; "dup": duplicate K/V
DEBUG_WT = False  # also emit per-pair exp(score) tiles to a debug output
GROUPS = [[0, 1], [2, 3], [4, 5], [6, 7]]
MM_DT = "bf16"  # kept for test.py compat; ignored

_prog_cache = {}


def _split_multi_waits(nc, max_waits=1):
    """walrus in this container rejects more than one sem wait per
    instruction ("Too many sync wait commands"). After Tile scheduling,
    hoist extra waits onto same-engine nops inserted just before the
    instruction (same blocking semantics: engine queues are in-order)."""
    from concourse import mybir

    n = 0
    for fn in nc.m.functions:
        for bb in fn.blocks:
            out = []
            for ins in bb.instructions:
                si = ins.sync_info
                waits = list(si.on_wait) if si and si.on_wait else []
                if len(waits) > max_waits:
                    extra = waits[:-max_waits]
                    si.on_wait = waits[-max_waits:]
                    for j in range(0, len(extra), max_waits):
                        nop = mybir.InstNoOp(
                            name=f"waitsplit_{n}", ins=[], outs=[],
                            engine=ins.engine)
                        n += 1
                        nop.sync_info = mybir.SyncInfo(
                            on_wait=extra[j:j + max_waits], on_update=[])
                        out.append(nop)
                out.append(ins)
            bb.instructions[:] = out


def _build_program(mode):
    import contextlib

    import concourse.bass as bass
    import concourse.tile as tile
    from concourse import mybir
    from concourse.masks import make_identity

    f32 = mybir.dt.float32
    bf16 = mybir.dt.bfloat16
    cc = mode == "cc"
    EH = 4 if cc else 8  # e-chunks of K/V computed locally

    nc = bass.Bass("TRN2", target_bir_lowering=False, debug=False,
                   num_devices=NCORES, dynamic_dma_scratch_size=2048)

    xqT_d = nc.dram_tensor("xqT", [D, 8 * P], bf16, kind="ExternalInput").ap()
    xkT_d = nc.dram_tensor("xkT", [D, N], bf16, kind="ExternalInput").ap()
    wq_d = nc.dram_tensor("wq", [D, D], bf16, kind="ExternalInput").ap()
    wk_d = nc.dram_tensor("wk", [D, EH * P], bf16, kind="ExternalInput").ap()
    wv_d = nc.dram_tensor("wv", [D, EH * P], bf16, kind="ExternalInput").ap()
    # 4 mask blocks: [cm0|cm1] 256-wide (A-zeros | B-mask at j=sB-2,sB-1)
    # then [am0|am1] 128-wide (A-mask at j=sA-2,sA-1)
    mask_d = nc.dram_tensor("mask", [P, 768], bf16, kind="ExternalInput").ap()
    out_d = nc.dram_tensor("out", [8 * P, D], f32, kind="ExternalOutput").ap()
    wtdbg_d = None
    if DEBUG_WT:
        wtdbg_d = nc.dram_tensor(
            "wtdbg", [4, P, 16, 2 * P], bf16, kind="ExternalOutput").ap()

    if cc:
        stg_k_in = nc.dram_tensor("stg_k_in", [4, P, N], bf16, kind="Internal")
        stg_k_out = nc.dram_tensor(
            "stg_k_out", [2, 4, P, N], bf16, kind="Internal")
        stg_v_in = nc.dram_tensor(
            "stg_v_in", [16, P, 4 * P], bf16, kind="Internal")
        stg_v_out = nc.dram_tensor(
            "stg_v_out", [2, 16, P, 4 * P], bf16, kind="Internal")

    xqT_r = xqT_d.rearrange("(dc p) q -> p dc q", p=P)
    xkT_r = xkT_d.rearrange("(dc p) k -> p dc k", p=P)
    wq_r = wq_d.rearrange("(dc p) e -> p dc e", p=P)
    wk_r = wk_d.rearrange("(dc p) e -> p dc e", p=P)
    wv_r = wv_d.rearrange("(dc p) e -> p dc e", p=P)

    with tile.TileContext(nc) as tc, contextlib.ExitStack() as ctx:
        cpool = ctx.enter_context(tc.tile_pool(name="cpool", bufs=1))
        qtp = ctx.enter_context(tc.tile_pool(name="qtp", bufs=1))
        ktp = ctx.enter_context(tc.tile_pool(name="ktp", bufs=1))
        vp = ctx.enter_context(tc.tile_pool(name="vp", bufs=1))

        ident_f = cpool.tile([P, P], f32, name="ident_f")
        make_identity(nc, ident_f)
        ident = cpool.tile([P, P], bf16, name="ident")
        nc.vector.tensor_copy(ident[:], ident_f[:])
        mask_sb = cpool.tile([P, 768], bf16, name="mask_sb")
        nc.sync.dma_start(mask_sb[:], mask_d)
        ones_sb = cpool.tile([P, 1], bf16, name="ones_sb")
        nc.vector.memset(ones_sb[:], 1.0)
        stats = cpool.tile([P, 8], f32, name="stats")

        QT = qtp.tile([P, DC, 8 * P], bf16, name="QT")
        KT = ktp.tile([P, DC, N], bf16, name="KT")
        V = vp.tile([P, N // P, D], bf16, name="V")

        # ---- projections ----
        with tc.tile_pool(name="xpool", bufs=4) as xpool, \
             tc.tile_pool(name="wpool", bufs=1) as wpool, \
             tc.tile_pool(name="hpool", bufs=1) as hpool, \
             tc.tile_pool(name="ppj", bufs=4, space="PSUM") as ppj:

            wq_sb = wpool.tile([P, DC, D], bf16, name="wq_sb")
            wk_sb = wpool.tile([P, DC, EH * P], bf16, name="wk_sb")
            wv_sb = wpool.tile([P, DC, EH * P], bf16, name="wv_sb")
            nc.sync.dma_start(wk_sb[:], wk_r)
            xk = []
            for kt in range(4):
                t = xpool.tile([P, DC, 4 * P], bf16, tag="x", name=f"xk{kt}")
                nc.sync.dma_start(t[:], xkT_r[:, :, kt * 512:(kt + 1) * 512])
                xk.append(t)
            nc.sync.dma_start(wv_sb[:], wv_r)
            nc.sync.dma_start(wq_sb[:], wq_r)

            # K^T[e, k] (own e-half in cc mode); kt-outer so the first
            # psum groups need only xk[0] while xk[1..3] stream in
            if cc:
                k_half = hpool.tile([P, 4, N], bf16, name="k_half")
                for kt in range(4):
                    for ec in range(4):
                        ps = ppj.tile([P, 512], f32, tag="pj", name="ps_k")
                        for dc in range(DC):
                            nc.tensor.matmul(
                                ps, wk_sb[:, dc, ec * P:(ec + 1) * P],
                                xk[kt][:, dc, :],
                                start=(dc == 0), stop=(dc == DC - 1))
                        nc.vector.tensor_copy(
                            k_half[:, ec, kt * 512:(kt + 1) * 512], ps)
                for ec in range(4):
                    nc.sync.dma_start(stg_k_in.ap()[ec], k_half[:, ec, :])
                nc.gpsimd.collective_compute(
                    "AllGather", mybir.AluOpType.bypass,
                    replica_groups=GROUPS,
                    ins=[stg_k_in.ap()], outs=[stg_k_out.ap()])
            else:
                for kt in range(4):
                    for ec in range(8):
                        ps = ppj.tile([P, 512], f32, tag="pj", name="ps_k")
                        for dc in range(DC):
                            nc.tensor.matmul(
                                ps, wk_sb[:, dc, ec * P:(ec + 1) * P],
                                xk[kt][:, dc, :],
                                start=(dc == 0), stop=(dc == DC - 1))
                        nc.vector.tensor_copy(
                            KT[:, ec, kt * 512:(kt + 1) * 512], ps)

            # V[k, e] (own e-half in cc mode)
            if cc:
                v_half = hpool.tile([P, 16, 4 * P], bf16, name="v_half")
                for kc in range(16):
                    ps = ppj.tile([P, 512], f32, tag="pj", name="ps_v")
                    xt = xk[kc // 4]
                    ko = (kc % 4) * P
                    for dc in range(DC):
                        nc.tensor.matmul(
                            ps, xt[:, dc, ko:ko + P], wv_sb[:, dc, :],
                            start=(dc == 0), stop=(dc == DC - 1))
                    nc.vector.tensor_copy(v_half[:, kc, :], ps)
                    nc.sync.dma_start(stg_v_in.ap()[kc], v_half[:, kc, :])
                nc.gpsimd.collective_compute(
                    "AllGather", mybir.AluOpType.bypass,
                    replica_groups=GROUPS,
                    ins=[stg_v_in.ap()], outs=[stg_v_out.ap()])
            else:
                for kc in range(16):
                    xt = xk[kc // 4]
                    ko = (kc % 4) * P
                    for h in range(2):
                        ps = ppj.tile([P, 512], f32, tag="pj", name="ps_v")
                        for dc in range(DC):
                            nc.tensor.matmul(
                                ps, xt[:, dc, ko:ko + P],
                                wv_sb[:, dc, h * 512:(h + 1) * 512],
                                start=(dc == 0), stop=(dc == DC - 1))
                        nc.vector.tensor_copy(
                            V[:, kc, h * 512:(h + 1) * 512], ps)

            # Q^T[e, q] for this core's 8 q-blocks (full d_out)
            for qc in range(2):
                xq = xpool.tile([P, DC, 4 * P], bf16, tag="x", name="xq")
                nc.sync.dma_start(
                    xq[:], xqT_r[:, :, qc * 512:(qc + 1) * 512])
                for ec in range(8):
                    ps = ppj.tile([P, 512], f32, tag="pj", name="ps_q")
                    for dc in range(DC):
                        nc.tensor.matmul(
                            ps, wq_sb[:, dc, ec * P:(ec + 1) * P],
                            xq[:, dc, :],
                            start=(dc == 0), stop=(dc == DC - 1))
                    nc.scalar.activation(
                        QT[:, ec, qc * 512:(qc + 1) * 512], ps,
                        mybir.ActivationFunctionType.Copy)

            # collective readbacks LAST in the SP stream: every earlier DMA
            # (staging, xq) must not queue behind a readback that waits on
            # the AllGather semaphore (SP is head-of-line blocking)
            if cc:
                for half in range(2):
                    for ec in range(4):
                        nc.sync.dma_start(
                            KT[:, half * 4 + ec, :], stg_k_out.ap()[half, ec])
                for kc in range(16):
                    for half in range(2):
                        nc.sync.dma_start(
                            V[:, kc, half * 512:(half + 1) * 512],
                            stg_v_out.ap()[half, kc])

        # ---- attention over 4 slot-pairs ----
        with tc.tile_pool(name="wtp", bufs=2) as wtp, \
             tc.tile_pool(name="obp", bufs=2) as obp, \
             tc.tile_pool(name="pst", bufs=2, space="PSUM") as pst, \
             tc.tile_pool(name="pav", bufs=4, space="PSUM") as pav, \
             tc.tile_pool(name="pdn", bufs=1, space="PSUM") as pdn:

            wts = [None] * 4
            dens = [None] * 4

            def emit_scores(i):
                sA, sB = SPAIRS[i]
                q0 = 2 * i * P  # pair's 256-wide q-range in QT
                wt = wtp.tile([P, 16, 2 * P], bf16, tag="wt", name=f"wt{i}")
                wts[i] = wt
                for j in range(sA):
                    wide = 2 * P if j < sB else P
                    ps = pst.tile([P, 2 * P], f32, tag="st", name=f"ps_s{i}")
                    # mask as an extra accumulation step (identity x mask)
                    if j == sB - 2:
                        mk = mask_sb[:, 0:256]
                    elif j == sB - 1:
                        mk = mask_sb[:, 256:512]
                    elif j == sA - 2:
                        mk = mask_sb[:, 512:640]
                    elif j == sA - 1:
                        mk = mask_sb[:, 640:768]
                    else:
                        mk = None
                    for ec in range(DC):
                        nc.tensor.matmul(
                            ps[:, :wide],
                            KT[:, ec, j * P:(j + 1) * P],
                            QT[:, ec, q0:q0 + wide],
                            start=(ec == 0),
                            stop=(ec == DC - 1 and mk is None))
                    if mk is not None:
                        nc.tensor.matmul(ps[:, :wide], ident[:], mk,
                                         start=False, stop=True)
                    nc.scalar.activation(
                        wt[:, j, 0:wide], ps[:, :wide],
                        mybir.ActivationFunctionType.Exp, scale=1.0 / 32.0)
                if DEBUG_WT:
                    nc.sync.dma_start(wtdbg_d[i], wt[:])

            def emit_av(i):
                sA, sB = SPAIRS[i]
                wt = wts[i]
                # one PSUM bank per denominator: a matmul group-start zeroes
                # the whole bank, so A and B groups must not share one
                den = [pdn.tile([P, 1], f32, tag="dnA", name=f"denA{i}"),
                       pdn.tile([P, 1], f32, tag="dnB", name=f"denB{i}")]
                dens[i] = den
                avs = [[None, None], [None, None]]  # [ab][h]
                for ab in range(2):
                    for h in range(2):
                        avs[ab][h] = pav.tile(
                            [P, 512], f32, tag="av", name=f"av{i}_{ab}{h}")
                for j in range(sA):
                    for ab in range(2):
                        s = (sA, sB)[ab]
                        if j >= s:
                            continue
                        wcol = ab * P
                        for h in range(2):
                            nc.tensor.matmul(
                                avs[ab][h], wt[:, j, wcol:wcol + P],
                                V[:, j, h * 512:(h + 1) * 512],
                                start=(j == 0), stop=(j == s - 1))
                        nc.tensor.matmul(
                            den[ab][:], wt[:, j, wcol:wcol + P],
                            ones_sb[:],
                            start=(j == 0), stop=(j == s - 1))
                ob = obp.tile([P, 2, D], f32, tag="ob", name=f"ob{i}")
                for ab in range(2):
                    sc = 2 * i + ab
                    nc.vector.reciprocal(
                        stats[:, sc:sc + 1], den[ab][:])
                    for h in range(2):
                        nc.vector.tensor_scalar_mul(
                            ob[:, ab, h * 512:(h + 1) * 512],
                            avs[ab][h], stats[:, sc:sc + 1])
                        nc.sync.dma_start(
                            out_d[sc * P:(sc + 1) * P,
                                  h * 512:(h + 1) * 512],
                            ob[:, ab, h * 512:(h + 1) * 512])

            emit_scores(0)
            for i in range(4):
                if i + 1 < 4:
                    emit_scores(i + 1)
                emit_av(i)

    _split_multi_waits(nc)
    return nc


def _host_prep(x, Wq, Wk, Wv):
    """Build per-core input maps (bf16 device inputs)."""
    import ml_dtypes
    bf = ml_dtypes.bfloat16

    x = np.ascontiguousarray(x, dtype=np.float32)
    ki = np.arange(P)[:, None]
    qi = np.arange(P)[None, :]
    triT = np.where(ki <= qi, 0.0, NEG).astype(np.float32)  # keep k <= q
    zero = np.zeros((P, P), np.float32)
    dead = np.full((P, P), NEG, np.float32)
    # blocks [cm0 | cm1 | am0 | am1]; cm* carry A-zeros in cols 0:128
    mask_r = {
        # parity 0: causal limit is odd -> diag then fully-dead block
        0: np.concatenate(
            [zero, triT, zero, dead, triT, dead], axis=1),
        # parity 1: causal limit is even -> full block then diag
        1: np.concatenate(
            [zero, zero, zero, triT, zero, triT], axis=1),
    }

    in_maps = []
    for c in range(NCORES):
        bi, r = c // 2, c % 2
        ecs = slice(r * 512, (r + 1) * 512) if MODE == "cc" else slice(0, D)
        rbs = [s - 2 + r for s in CAPS]
        xq = np.concatenate(
            [x[bi, rb * P:(rb + 1) * P, :] for rb in rbs], axis=0)
        in_maps.append({
            "xqT": np.ascontiguousarray(xq.T).astype(bf),
            "xkT": np.ascontiguousarray(x[bi].T).astype(bf),
            "wq": np.ascontiguousarray(Wq).astype(bf),
            "wk": np.ascontiguousarray(Wk[:, ecs]).astype(bf),
            "wv": np.ascontiguousarray(Wv[:, ecs]).astype(bf),
            "mask": np.ascontiguousarray(mask_r[r]).astype(bf),
        })
    return in_maps


def _host_gather(results):
    out = np.empty((B, N, D), dtype=np.float32)
    for c in range(NCORES):
        bi, r = c // 2, c % 2
        res = results[c]["out"]
        for t, s in enumerate(CAPS):
            rb = s - 2 + r
            out[bi, rb * P:(rb + 1) * P, :] = res[t * P:(t + 1) * P, :]
    return out


def kernel(x, Wq, Wk, Wv, _trace=False, _trace_kwargs=None):
    from concourse.bass_utils import run_bass_kernel_spmd

    key = MODE
    if key not in _prog_cache:
        _prog_cache[key] = _build_program(key)
    nc = _prog_cache[key]

    in_maps = _host_prep(x, Wq, Wk, Wv)
    kw = dict(_trace_kwargs or {})
    res = run_bass_kernel_spmd(nc, in_maps, list(range(NCORES)),
                               trace=_trace, **kw)
    out = _host_gather(res.results)
    if _trace:
        return out, res
    return out
